# revision 11
# baseline (speedup 1.0000x reference)
"""Trainium2 Bass kernel for nn_CanineAttention (block-diagonal local attention).

Reference computation (per batch b):
  q/k/v = x @ W{q,k,v} + b{q,k,v}            x: [B,S,H]=[4,4096,768]
  per chunk of W=128 tokens, per head (NH=12, HD=64):
    scores = q k^T / 8 + (1-mask_diag)*(-1e4); probs = softmax(scores)
    ctx = probs @ v
  out = LayerNorm(ctx @ Wo + bo + x) * gamma + beta
Sharding: tokens (B*S = 16384) split contiguously across 8 cores
(2048 tokens = 16 chunks per core), fully data-parallel, no collectives.

Device layout notes:
  - Projections run with the contraction dim (h_in) on partitions; the kernel
    consumes x TRANSPOSED (host pre-transposes to [H, tokens]).
  - qT, kT are produced in [h_out, t] layout (head dims on partitions).
  - Attention (T2 layout, modes fp16/bf16): scores are computed TRANSPOSED
    ([k, q]) by swapping the operands, exp'd on ACT, and PV runs in natural
    layout (out [q, d]) with a fused ones-column in the V tile producing the
    softmax denominator as a 65th output column. Normalization is then a
    per-partition scaled copy on ACT. Natural ctx is transposed back on the
    PE (6 transposes/chunk instead of 12) for the output projection.
  - V is stored as 12 groups of 65 columns (64 dims + a ones column).
  - Softmax skips max-subtraction (scores are O(1); the additive mask bias
    is <= 0 so exp() only underflows, never overflows).
  - LayerNorm: residual is added during the PSUM->SBUF copy (DVE), stats via
    bn_stats/bn_aggr, rstd via bit-trick + 2 Newton steps, apply on ACT with
    per-partition scale/bias.
  - One PSUM pool, one tag, bufs=8: every PSUM tile is a full bank; rotation
    guarantees concurrent quadrant matmuls land in distinct banks.
  - Precision modes: "f32"/"split3"/... use a legacy attention path; the
    default "fp16" runs everything at 1 cycle/row on the PE.
"""

import numpy as np
import ml_dtypes
from contextlib import ExitStack

import concourse.bass as bass
import concourse.tile as tile
from concourse import bacc, mybir
from concourse.bass_utils import run_bass_kernel_spmd
from concourse.masks import make_identity

# ---------------- problem constants (hardcoded per contract) ----------------
B, S, H, NH, W = 4, 4096, 768, 12, 128
HD = H // NH            # 64
C = S // W              # 32 chunks
NEG = -10000.0
EPS = 1e-12

NCORES = 8
TPC = B * S // NCORES   # 2048 tokens per core
CPC = TPC // W          # 16 chunks per core
BLK = 512               # tokens per processing block
NBLK = TPC // BLK       # 4 blocks
CPB = BLK // W          # 4 chunks per block
NG = H // 128           # 6 partition-chunks over H
NPAIR = NH // 2         # 6 head pairs (2 heads x 64 dims = 128 partitions)

F32 = mybir.dt.float32
BF16 = mybir.dt.bfloat16
FP = mybir.ActivationFunctionType
OP = mybir.AluOpType
AX = mybir.AxisListType

MODE = "fp16"   # default precision mode; see module docstring

F16 = mybir.dt.float16
MODES = {
    # dt: matmul operand dtype; xsplit: x = hi+lo; wsplit: W = hi+lo
    "f32":    dict(dt=F32,  xsplit=False, wsplit=False),
    "bf16":   dict(dt=BF16, xsplit=False, wsplit=False),
    "fp16":   dict(dt=F16,  xsplit=False, wsplit=False),
    "fp16s2": dict(dt=F16,  xsplit=False, wsplit=True),
    "split3": dict(dt=BF16, xsplit=True,  wsplit=True),
    "fp16s3": dict(dt=F16,  xsplit=True,  wsplit=True),
}


def _is_t2(mode):
    """T2 attention layout: only for 1-pass 16-bit modes."""
    return mode in ("fp16", "bf16")


# ---------------------------------------------------------------------------
# kernel builder
# ---------------------------------------------------------------------------

def _build(mode, use_mask, use_qbias, use_kbias, use_vbias, use_ln_affine, reps=1):
    """Build + compile the SPMD single-core program. Returns (nc, input_names).

    reps>1 repeats the whole computation (idempotent output writes) so HW
    kernel time can be measured as the slope over reps."""
    nc = bacc.Bacc(
        "TRN2", target_bir_lowering=False, debug=False,
        enable_asserts=False, num_devices=NCORES,
    )

    xsplit, wsplit = MODES[mode]["xsplit"], MODES[mode]["wsplit"]
    wdt = MODES[mode]["dt"]
    xdt = wdt
    t2 = _is_t2(mode)
    # attention matmul operand dtype
    adt = wdt if t2 else (F16 if mode == "fp16" else F32)

    # ---------------- DRAM I/O ----------------
    names = []

    def dram_in(name, shape, dt):
        names.append(name)
        return nc.dram_tensor(name, shape, dt, kind="ExternalInput").ap()

    xt_hi = dram_in("xt_hi", [H, TPC], xdt)            # x^T (hi part if split)
    xt_lo = dram_in("xt_lo", [H, TPC], xdt) if xsplit else None
    xres = dram_in("xres", [TPC, H], F32)              # x + bo (residual input)
    w_dram = {"wq": dram_in("wq_hi", [H, H], wdt),
              "wk": dram_in("wk_hi", [H, H], wdt),
              "wv": dram_in("wv_hi", [H, H], wdt),
              "wo": dram_in("wo_hi", [H, H], wdt)}
    if wsplit:
        for wn in ("wq", "wk", "wv", "wo"):
            w_dram[wn + "_lo"] = dram_in(wn + "_lo", [H, H], wdt)
    bq = dram_in("bq", [128, NG], F32) if use_qbias else None   # (bq/8) chunked
    bk = dram_in("bk", [128, NG], F32) if use_kbias else None
    bvb = dram_in("bvb", [128, H], F32) if use_vbias else None  # bv broadcast
    gmb = dram_in("gmb", [128, H], F32) if use_ln_affine else None
    btb = dram_in("btb", [128, H], F32) if use_ln_affine else None
    mbias = dram_in("mbias", [CPC, W, W], F32) if use_mask else None
    out = nc.dram_tensor("out", [TPC, H], F32, kind="ExternalOutput").ap()

    # matmul pass list: (x-half, weight-key)
    if xsplit and wsplit:
        passes = {wn: [("hi", wn), ("hi", wn + "_lo"), ("lo", wn)]
                  for wn in ("wq", "wk", "wv", "wo")}
    elif wsplit:
        passes = {wn: [("hi", wn), ("hi", wn + "_lo")]
                  for wn in ("wq", "wk", "wv", "wo")}
    else:
        passes = {wn: [("hi", wn)] for wn in ("wq", "wk", "wv", "wo")}

    with tile.TileContext(nc) as tc, ExitStack() as ctx:
        const = ctx.enter_context(tc.tile_pool(name="const", bufs=1))
        xp = ctx.enter_context(tc.tile_pool(name="xp", bufs=2))
        qkv = ctx.enter_context(tc.tile_pool(name="qkv", bufs=1))
        attn = ctx.enter_context(tc.tile_pool(name="attn", bufs=3))
        ctxnp = ctx.enter_context(tc.tile_pool(name="ctxnp", bufs=2))
        ctxp = ctx.enter_context(tc.tile_pool(name="ctxp", bufs=1))
        outp = ctx.enter_context(tc.tile_pool(name="outp", bufs=2))
        # single PSUM pool: every tile = one bank, one shared tag, 8-deep
        # rotation => concurrent quadrant matmuls always hit distinct banks
        pp = ctx.enter_context(tc.tile_pool(name="pp", bufs=8, space="PSUM"))

        def ptile(shape, dt, name=None):
            return pp.tile(shape, dt, tag="ps", name=name)

        # ---------------- constants ----------------
        ident = const.tile([128, 128], adt, tag="ident")
        make_identity(nc, ident)

        w_sb = {}   # key -> list of NG chunk tiles [128, H]
        for wn, ap in w_dram.items():
            w_sb[wn] = []
            for g in range(NG):
                t = const.tile([128, H], wdt, tag=f"{wn}{g}")
                nc.sync.dma_start(t[:], ap[g * 128:(g + 1) * 128, :])
                w_sb[wn].append(t)

        bq_sb = bk_sb = bvb_sb = gmb_sb = btb_sb = None
        if use_qbias:
            bq_sb = const.tile([128, NG], F32, tag="bq")
            nc.sync.dma_start(bq_sb[:], bq)
        if use_kbias:
            bk_sb = const.tile([128, NG], F32, tag="bk")
            nc.sync.dma_start(bk_sb[:], bk)
        if use_vbias:
            bvb_sb = const.tile([128, H], F32, tag="bvb")
            nc.sync.dma_start(bvb_sb[:], bvb)
        if use_ln_affine:
            gmb_sb = const.tile([128, H], F32, tag="gmb")
            nc.sync.dma_start(gmb_sb[:], gmb)
            btb_sb = const.tile([128, H], F32, tag="btb")
            nc.sync.dma_start(btb_sb[:], btb)

        # ---------------- per token-block ----------------
        # reps>1: device-side hardware loop repeating the whole computation
        # (for slope-based HW timing); body is identical each iteration.
        import contextlib
        rep_cm = tc.For_i(0, reps, 1) if reps > 1 else contextlib.nullcontext()
        with rep_cm:
          for blk in range(NBLK):
            t0 = blk * BLK

            # -- load x^T block --
            xth = [xp.tile([128, BLK], xdt, tag=f"xth{g}", name=f"xth{g}") for g in range(NG)]
            for g in range(NG):
                nc.sync.dma_start(xth[g][:], xt_hi[g * 128:(g + 1) * 128, t0:t0 + BLK])
            if xsplit:
                xtl = [xp.tile([128, BLK], xdt, tag=f"xtl{g}", name=f"xtl{g}") for g in range(NG)]
                for g in range(NG):
                    nc.sync.dma_start(xtl[g][:], xt_lo[g * 128:(g + 1) * 128, t0:t0 + BLK])

            def xop(sel, g):
                return xth[g] if sel == "hi" else xtl[g]

            # -- Q/K projections (transposed layout [h_out, t]) --
            qT, kT = [], []
            for which, wn, dst in (("q", "wq", qT), ("k", "wk", kT)):
                for go in range(NG):
                    ps = ptile([128, BLK], F32, name="proj")
                    mms = [(wkey, gi, xsel) for xsel, wkey in passes[wn] for gi in range(NG)]
                    for i, (wkey, gi, xsel) in enumerate(mms):
                        nc.tensor.matmul(
                            ps[:],
                            w_sb[wkey][gi][:, go * 128:(go + 1) * 128],
                            xop(xsel, gi)[:],
                            start=(i == 0), stop=(i == len(mms) - 1),
                        )
                    sb = qkv.tile([128, BLK], adt, tag=f"{which}T{go}")
                    scl = 0.125 if which == "q" else 1.0
                    has_b = use_qbias if which == "q" else use_kbias
                    if has_b:
                        bias = (bq_sb if which == "q" else bk_sb)[:, go:go + 1]
                        nc.scalar.activation(sb[:], ps[:], FP.Identity, bias=bias, scale=scl)
                    else:
                        nc.scalar.activation(sb[:], ps[:], FP.Copy, scale=scl)
                    dst.append(sb)

            # -- V projection --
            vN = []
            if t2:
                # natural layout in 12 groups of 65 cols (64 dims + ones col)
                for tt in range(CPB):
                    vt = qkv.tile([128, NH * 65], adt, tag=f"v{tt}")
                    v3 = vt[:].rearrange("p (g c) -> p g c", c=65)
                    nc.vector.memset(v3[:, :, 64:65], 1.0)
                    for nhalf in range(2):
                        n0 = nhalf * 384
                        ps = ptile([128, 384], F32, name="proj")
                        mms = [(wkey, gi, xsel) for xsel, wkey in passes["wv"] for gi in range(NG)]
                        for i, (wkey, gi, xsel) in enumerate(mms):
                            nc.tensor.matmul(
                                ps[:],
                                xop(xsel, gi)[:, tt * 128:(tt + 1) * 128],
                                w_sb[wkey][gi][:, n0:n0 + 384],
                                start=(i == 0), stop=(i == len(mms) - 1),
                            )
                        dstv = v3[:, 6 * nhalf:6 * nhalf + 6, 0:64]
                        srcv = ps[:].rearrange("p (g c) -> p g c", c=64)
                        if use_vbias:
                            bsrc = bvb_sb[:, n0:n0 + 384].rearrange("p (g c) -> p g c", c=64)
                            nc.vector.tensor_add(dstv, srcv, bsrc)
                        else:
                            nc.vector.tensor_copy(dstv, srcv)
                    vN.append(vt)
            else:
                for tt in range(CPB):
                    vt = qkv.tile([128, H], adt, tag=f"v{tt}")
                    for nhalf in range(2):
                        n0 = nhalf * 384
                        ps = ptile([128, 384], F32, name="proj")
                        mms = [(wkey, gi, xsel) for xsel, wkey in passes["wv"] for gi in range(NG)]
                        for i, (wkey, gi, xsel) in enumerate(mms):
                            nc.tensor.matmul(
                                ps[:],
                                xop(xsel, gi)[:, tt * 128:(tt + 1) * 128],
                                w_sb[wkey][gi][:, n0:n0 + 384],
                                start=(i == 0), stop=(i == len(mms) - 1),
                            )
                        if use_vbias:
                            nc.vector.tensor_add(vt[:, n0:n0 + 384], ps[:], bvb_sb[:, n0:n0 + 384])
                        else:
                            nc.vector.tensor_copy(vt[:, n0:n0 + 384], ps[:])
                    vN.append(vt)

            # -- block-diagonal attention --
            cxdt = F32 if mode == "f32" else wdt
            if t2:
                # cxall [128, NG*BLK]: head-pair g occupies cols g*BLK..(g+1)*BLK
                cxall = ctxp.tile([128, NG * BLK], cxdt, tag="cxall")
                cxh = None
            else:
                cxh = [ctxp.tile([128, BLK], cxdt, tag=f"cxh{g}", name=f"cxh{g}") for g in range(NG)]
            cxl = [ctxp.tile([128, BLK], wdt, tag=f"cxl{g}", name=f"cxl{g}") for g in range(NG)] if wsplit else None

            if t2:
                # ---- T2 path: transposed scores, natural-ctx PV, software-
                # pipelined two chunks deep so PE never waits on the in-order
                # ACT/DVE queues:
                #   iter cc: [pass2(cc-1), pass1(cc), transpose(cc-1),
                #             out-proj+LN(cc-2)]
                def pass1(cc):
                    ts = slice(cc * 128, (cc + 1) * 128)
                    if use_mask:
                        mb = attn.tile([128, W], F32, tag="mb")
                        nc.sync.dma_start(mb[:], mbias[blk * CPB + cc])
                    exs = []
                    for g in range(NPAIR):
                        ps_s = [ptile([128, W], F32, name=f"scps{h}")
                                for h in range(2)]
                        for h in range(2):
                            p0 = h * 64
                            # scoresT[k, q] = kT_h^T @ qT_h
                            nc.tensor.matmul(
                                ps_s[h][:],
                                kT[g][p0:p0 + 64, ts],
                                qT[g][p0:p0 + 64, ts],
                                start=True, stop=True,
                                tile_position=(p0, 0),
                            )
                        ex = attn.tile([128, 2 * W], adt, tag="ex", bufs=12)
                        for h in range(2):
                            src = ps_s[h][:]
                            if use_mask:
                                sm = attn.tile([128, W], F32, tag="sm")
                                nc.vector.tensor_add(sm[:], src, mb[:])
                                src = sm[:]
                            nc.scalar.activation(ex[:, h * W:(h + 1) * W], src, FP.Exp)
                        exs.append(ex)
                    return exs

                def pass2(cc, exs):
                    ctxn = ctxnp.tile([128, H], adt, tag="ctxn")
                    for g in range(NPAIR):
                        # PV natural: pc[q, 65] per head; col 64 = denominator
                        pc = ptile([128, 130], F32, name="pc")
                        for h in range(2):
                            hh = 2 * g + h
                            nc.tensor.matmul(
                                pc[:, h * 65:(h + 1) * 65],
                                exs[g][:, h * W:(h + 1) * W],
                                vN[cc][:, hh * 65:(hh + 1) * 65],
                                start=True, stop=True,
                                skip_group_check=(h == 1),
                            )
                        rc = attn.tile([128, 2], F32, tag="rc")
                        den3 = pc[:].rearrange("p (h c) -> p h c", c=65)[:, :, 64:65]
                        rc3 = rc[:].rearrange("p (h o) -> p h o", o=1)
                        nc.vector.reciprocal(rc3, den3)
                        # normalize: head 0 on DVE, head 1 on ACT (balance)
                        nc.vector.tensor_scalar_mul(
                            ctxn[:, (2 * g) * 64:(2 * g + 1) * 64],
                            pc[:, 0:64], rc[:, 0:1])
                        nc.scalar.activation(
                            ctxn[:, (2 * g + 1) * 64:(2 * g + 2) * 64],
                            pc[:, 65:129], FP.Identity, scale=rc[:, 1:2])
                    return ctxn

                def do_tp(cc, ctxn):
                    ts = slice(cc * 128, (cc + 1) * 128)
                    # transpose natural ctx back to [hd, t]: 6 transposes into
                    # one fp16 PSUM bank, then one strided DVE copy
                    tp = ptile([128, H], adt, name="tp")
                    for g in range(NG):
                        nc.tensor.matmul(
                            tp[:, g * 128:(g + 1) * 128],
                            ctxn[:, g * 128:(g + 1) * 128],
                            ident[:], is_transpose=True,
                            skip_group_check=(g > 0),
                        )
                    dstc = cxall[:].rearrange("p (g t) -> p g t", g=NG)[:, :, ts]
                    srcc = tp[:].rearrange("p (g c) -> p g c", c=128)
                    nc.vector.tensor_copy(dstc, srcc)
            else:
              for cc in range(CPB):
                ts = slice(cc * 128, (cc + 1) * 128)
                chunk_idx = blk * CPB + cc
                if use_mask:
                    mb = attn.tile([128, W], F32, tag="mb")
                    nc.sync.dma_start(mb[:], mbias[chunk_idx])
                if True:
                    # ---- legacy path: scores [q, k], PE prob transpose ----
                    for g in range(NPAIR):
                        ps_s = [ptile([128, W], F32, name=f"scps{h}")
                                for h in range(2)]
                        for h in range(2):
                            p0 = h * 64
                            nc.tensor.matmul(
                                ps_s[h][:],
                                qT[g][p0:p0 + 64, ts],
                                kT[g][p0:p0 + 64, ts],
                                start=True, stop=True,
                                tile_position=(p0, 0),
                            )
                        den = attn.tile([128, 2], F32, tag="den")
                        ex = attn.tile([128, 2 * W], F32, tag="ex")
                        for h in range(2):
                            src = ps_s[h][:]
                            if use_mask:
                                sm = attn.tile([128, W], F32, tag="sm")
                                nc.vector.tensor_add(sm[:], src, mb[:])
                                src = sm[:]
                            nc.scalar.activation(
                                ex[:, h * W:(h + 1) * W], src, FP.Exp,
                                accum_out=den[:, h:h + 1],
                            )
                        rec = attn.tile([128, 2], F32, tag="rec")
                        nc.vector.reciprocal(rec[:], den[:])
                        pr = attn.tile([128, 2 * W], adt, tag="pr")
                        for h in range(2):
                            nc.vector.tensor_scalar_mul(
                                pr[:, h * W:(h + 1) * W], ex[:, h * W:(h + 1) * W],
                                rec[:, h:h + 1],
                            )
                        ps_t = ptile([128, 2 * W], adt, name="pt")
                        for h in range(2):
                            nc.tensor.matmul(
                                ps_t[:, h * W:(h + 1) * W], pr[:, h * W:(h + 1) * W],
                                ident[:], is_transpose=True,
                                skip_group_check=(h == 1),
                            )
                        pts = attn.tile([128, 2 * W], adt, tag="pts")
                        nc.vector.tensor_copy(pts[:], ps_t[:])
                        ps_c = ptile([128, W], F32, name="cx")
                        for h in range(2):
                            hd0 = (2 * g + h) * HD
                            nc.tensor.matmul(
                                ps_c[h * 64:(h + 1) * 64, :],
                                vN[cc][:, hd0:hd0 + HD],
                                pts[:, h * W:(h + 1) * W],
                                start=True, stop=True,
                                tile_position=(0, h * 64),
                                skip_group_check=(h == 1),
                            )
                        if wsplit:
                            nc.scalar.activation(cxh[g][:, ts], ps_c[:], FP.Copy)
                            nc.vector.tensor_sub(cxl[g][:, ts], ps_c[:], cxh[g][:, ts])
                        else:
                            nc.vector.tensor_copy(cxh[g][:, ts], ps_c[:])

            # -- output projection + residual + LayerNorm --
            if t2:
                def olhs(pi, gi, tt):
                    return cxall[:, gi * BLK + tt * 128:gi * BLK + (tt + 1) * 128]
                opasses = [0]
            else:
                if wsplit:
                    ocx = [(cxh, "wo"), (cxh, "wo_lo"), (cxl, "wo")]
                else:
                    ocx = [(cxh, "wo")]

                def olhs(pi, gi, tt):
                    return ocx[pi][0][gi][:, tt * 128:(tt + 1) * 128]
                opasses = list(range(len(ocx)))

            def owkey(pi):
                if t2:
                    return "wo"
                return ocx[pi][1]

            def oproj_ln(tt):
                r0 = t0 + tt * 128
                xr = outp.tile([128, H], F32, tag="xr")
                nc.sync.dma_start(xr[:], xres[r0:r0 + 128, :])
                hsb = outp.tile([128, H], F32, tag="hsb")
                for nhalf in range(2):
                    n0 = nhalf * 384
                    ps = ptile([128, 384], F32, name="ops")
                    mms = [(pi, gi) for pi in opasses for gi in range(NG)]
                    for i, (pi, gi) in enumerate(mms):
                        nc.tensor.matmul(
                            ps[:],
                            olhs(pi, gi, tt),
                            w_sb[owkey(pi)][gi][:, n0:n0 + 384],
                            start=(i == 0), stop=(i == len(mms) - 1),
                        )
                    nc.vector.tensor_add(hsb[:, n0:n0 + 384], ps[:], xr[:, n0:n0 + 384])

                # LayerNorm stats via bn_stats/bn_aggr
                st = outp.tile([128, 12], F32, tag="st")
                for nhalf in range(2):
                    nc.vector.bn_stats(st[:, nhalf * 6:(nhalf + 1) * 6],
                                       hsb[:, nhalf * 384:(nhalf + 1) * 384])
                mv = outp.tile([128, 2], F32, tag="mv")
                nc.vector.bn_aggr(mv[:], st[:])
                var1 = outp.tile([128, 1], F32, tag="var1")
                nc.vector.tensor_scalar_add(var1[:], mv[:, 1:2], EPS)
                # rstd = 1/sqrt(var): bit-trick seed + 2 Newton steps (on DVE,
                # avoiding the ACT sqrt table-set switch and its poor ULP)
                rstd = outp.tile([128, 1], F32, tag="rstd")
                t1 = outp.tile([128, 1], F32, tag="t1n")
                ri = rstd[:].bitcast(mybir.dt.int32)
                nc.vector.tensor_scalar(
                    ri, var1[:].bitcast(mybir.dt.int32), 1, None,
                    op0=OP.logical_shift_right,
                )
                nc.vector.tensor_scalar(ri, ri, -1, 0x5F3759DF, op0=OP.mult, op1=OP.add)
                for _ in range(2):
                    nc.vector.tensor_mul(t1[:], rstd[:], rstd[:])
                    nc.vector.tensor_mul(t1[:], t1[:], var1[:])
                    nc.vector.tensor_scalar(t1[:], t1[:], -0.5, 1.5, op0=OP.mult, op1=OP.add)
                    nc.vector.tensor_mul(rstd[:], rstd[:], t1[:])
                # apply on ACT: out = (h - mu) * rstd = h*rstd + (-mu*rstd)
                nmr = outp.tile([128, 1], F32, tag="nmr")
                nc.vector.tensor_scalar(nmr[:], mv[:, 0:1], rstd[:], -1.0,
                                        op0=OP.mult, op1=OP.mult)
                ot = outp.tile([128, H], F32, tag="ot")
                for nhalf in range(2):
                    n0 = nhalf * 384
                    nc.scalar.activation(ot[:, n0:n0 + 384], hsb[:, n0:n0 + 384],
                                         FP.Identity, bias=nmr[:], scale=rstd[:])
                if use_ln_affine:
                    nc.vector.tensor_mul(ot[:], ot[:], gmb_sb[:])
                    nc.vector.tensor_add(ot[:], ot[:], btb_sb[:])
                nc.sync.dma_start(out[r0:r0 + 128, :], ot[:])

            if t2:
                exs_prev = None
                ctxn_prev = None
                for cc in range(CPB):
                    if exs_prev is not None:
                        ctxn_prev = pass2(cc - 1, exs_prev)
                    exs_new = pass1(cc)
                    if exs_prev is not None:
                        do_tp(cc - 1, ctxn_prev)
                    if cc >= 2:
                        oproj_ln(cc - 2)
                    exs_prev = exs_new
                ctxn_prev = pass2(CPB - 1, exs_prev)
                do_tp(CPB - 1, ctxn_prev)
                oproj_ln(CPB - 2)
                oproj_ln(CPB - 1)
            else:
                for tt in range(CPB):
                    oproj_ln(tt)

    nc.compile()
    return nc, names


# ---------------------------------------------------------------------------
# host-side wrapper
# ---------------------------------------------------------------------------

_CACHE = {}


def _get_program(mode, use_mask, use_qbias, use_kbias, use_vbias, use_ln_affine, reps=1):
    key = (mode, use_mask, use_qbias, use_kbias, use_vbias, use_ln_affine, reps)
    if key not in _CACHE:
        _CACHE[key] = _build(*key[:-1], reps=reps)
    return _CACHE[key]


def _prep_inputs(inputs, mode):
    """Host preprocessing -> per-core in_maps + program flags."""
    hs = np.ascontiguousarray(np.asarray(inputs["hidden_states"], dtype=np.float32))
    mask = np.asarray(inputs["attention_mask"], dtype=np.float32)
    Wq = np.asarray(inputs["Wq"], np.float32); bq = np.asarray(inputs["bq"], np.float32)
    Wk = np.asarray(inputs["Wk"], np.float32); bk = np.asarray(inputs["bk"], np.float32)
    Wv = np.asarray(inputs["Wv"], np.float32); bv = np.asarray(inputs["bv"], np.float32)
    Wo = np.asarray(inputs["Wo"], np.float32); bo = np.asarray(inputs["bo"], np.float32)
    gm = np.asarray(inputs["ln_gamma"], np.float32)
    bt = np.asarray(inputs["ln_beta"], np.float32)

    cfg = MODES[mode]
    xsplit, wsplit = cfg["xsplit"], cfg["wsplit"]
    npdt = {F32: np.float32, BF16: ml_dtypes.bfloat16, F16: np.float16}[cfg["dt"]]
    use_mask = not np.all(mask == 1.0)
    use_qbias = bool(np.any(bq)); use_kbias = bool(np.any(bk))
    use_vbias = bool(np.any(bv))
    use_ln_affine = bool(np.any(gm != 1.0) or np.any(bt))

    x = hs.reshape(B * S, H)
    xres_full = x + bo[None, :] if np.any(bo) else x

    def wpack(w):
        wh = w.astype(npdt)
        d = {"hi": np.ascontiguousarray(wh)}
        if wsplit:
            d["lo"] = np.ascontiguousarray((w - wh.astype(np.float32)).astype(npdt))
        return d

    wq, wk, wv, wo = wpack(Wq), wpack(Wk), wpack(Wv), wpack(Wo)

    if use_mask:
        # per-core diagonal [W,W] blocks of the mask -> additive bias
        m4 = mask.reshape(B, C, W, C, W)
        idx = np.arange(C)
        mblk = m4[:, idx, :, idx, :]                 # [C,B,W,W]
        mblk = np.transpose(mblk, (1, 0, 2, 3))      # [B,C,W,W]
        bias_blocks = ((1.0 - mblk) * NEG).astype(np.float32).reshape(B * C, W, W)
        if _is_t2(mode):
            # T2 computes scores transposed ([k, q]) -> transpose the bias
            bias_blocks = np.ascontiguousarray(np.transpose(bias_blocks, (0, 2, 1)))

    in_maps = []
    for c in range(NCORES):
        sl = x[c * TPC:(c + 1) * TPC]                # [TPC, H]
        m = {}
        xh = sl.astype(npdt)
        m["xt_hi"] = np.ascontiguousarray(xh.T)
        if xsplit:
            m["xt_lo"] = np.ascontiguousarray((sl - xh.astype(np.float32)).astype(npdt).T)
        m["xres"] = np.ascontiguousarray(xres_full[c * TPC:(c + 1) * TPC])
        for wn, d in (("wq", wq), ("wk", wk), ("wv", wv), ("wo", wo)):
            m[wn + "_hi"] = d["hi"]
            if wsplit:
                m[wn + "_lo"] = d["lo"]
        if use_qbias:
            m["bq"] = np.ascontiguousarray((bq / 8.0).reshape(NG, 128).T)
        if use_kbias:
            m["bk"] = np.ascontiguousarray(bk.reshape(NG, 128).T)
        if use_vbias:
            m["bvb"] = np.ascontiguousarray(np.broadcast_to(bv, (128, H)))
        if use_ln_affine:
            m["gmb"] = np.ascontiguousarray(np.broadcast_to(gm, (128, H)))
            m["btb"] = np.ascontiguousarray(np.broadcast_to(bt, (128, H)))
        if use_mask:
            m["mbias"] = np.ascontiguousarray(bias_blocks[c * CPC:(c + 1) * CPC])
        in_maps.append(m)

    flags = (use_mask, use_qbias, use_kbias, use_vbias, use_ln_affine)
    return in_maps, flags


def run(inputs, mode=None, trace=False, reps=1):
    """Run the kernel; returns (output [B,S,H] f32, BassKernelResults)."""
    mode = mode or MODE
    in_maps, flags = _prep_inputs(inputs, mode)
    nc, names = _get_program(mode, *flags, reps=reps)
    in_maps = [{k: v for k, v in m.items() if k in names} for m in in_maps]
    res = run_bass_kernel_spmd(nc, in_maps, list(range(NCORES)), trace=trace)
    outs = [res.results[c]["out"] for c in range(NCORES)]
    full = np.concatenate(outs, axis=0).reshape(B, S, H).astype(np.float32)
    return full, res


def kernel(**inputs):
    out, _ = run(inputs)
    return out


# revision 13
# speedup vs baseline: 1.0396x; 1.0396x over previous
"""Trainium2 Bass kernel for nn_CanineAttention (block-diagonal local attention).

Reference computation (per batch b):
  q/k/v = x @ W{q,k,v} + b{q,k,v}            x: [B,S,H]=[4,4096,768]
  per chunk of W=128 tokens, per head (NH=12, HD=64):
    scores = q k^T / 8 + (1-mask_diag)*(-1e4); probs = softmax(scores)
    ctx = probs @ v
  out = LayerNorm(ctx @ Wo + bo + x) * gamma + beta
Sharding: tokens (B*S = 16384) split contiguously across 8 cores
(2048 tokens = 16 chunks per core), fully data-parallel, no collectives.

Device layout notes:
  - Projections run with the contraction dim (h_in) on partitions; the kernel
    consumes x TRANSPOSED (host pre-transposes to [H, tokens]).
  - qT, kT are produced in [h_out, t] layout (head dims on partitions).
  - Attention (T2 layout, modes fp16/bf16): scores are computed TRANSPOSED
    ([k, q]) by swapping the operands, exp'd on ACT, and PV runs in natural
    layout (out [q, d]) with a fused ones-column in the V tile producing the
    softmax denominator as a 65th output column. Normalization is then a
    per-partition scaled copy on ACT. Natural ctx is transposed back on the
    PE (6 transposes/chunk instead of 12) for the output projection.
  - V is stored as 12 groups of 65 columns (64 dims + a ones column).
  - Softmax skips max-subtraction (scores are O(1); the additive mask bias
    is <= 0 so exp() only underflows, never overflows).
  - LayerNorm: residual is added during the PSUM->SBUF copy (DVE), stats via
    bn_stats/bn_aggr, rstd via bit-trick + 2 Newton steps, apply on ACT with
    per-partition scale/bias.
  - One PSUM pool, one tag, bufs=8: every PSUM tile is a full bank; rotation
    guarantees concurrent quadrant matmuls land in distinct banks.
  - Precision modes: "f32"/"split3"/... use a legacy attention path; the
    default "fp16" runs everything at 1 cycle/row on the PE.
"""

import numpy as np
import ml_dtypes
from contextlib import ExitStack

import concourse.bass as bass
import concourse.tile as tile
from concourse import bacc, mybir
from concourse.bass_utils import run_bass_kernel_spmd
from concourse.masks import make_identity

# ---------------- problem constants (hardcoded per contract) ----------------
B, S, H, NH, W = 4, 4096, 768, 12, 128
HD = H // NH            # 64
C = S // W              # 32 chunks
NEG = -10000.0
EPS = 1e-12

NCORES = 8
TPC = B * S // NCORES   # 2048 tokens per core
CPC = TPC // W          # 16 chunks per core
BLK = 512               # tokens per processing block
NBLK = TPC // BLK       # 4 blocks
CPB = BLK // W          # 4 chunks per block
NG = H // 128           # 6 partition-chunks over H
NPAIR = NH // 2         # 6 head pairs (2 heads x 64 dims = 128 partitions)

F32 = mybir.dt.float32
BF16 = mybir.dt.bfloat16
FP = mybir.ActivationFunctionType
OP = mybir.AluOpType
AX = mybir.AxisListType

MODE = "fp16"   # default precision mode; see module docstring

F16 = mybir.dt.float16
MODES = {
    # dt: matmul operand dtype; xsplit: x = hi+lo; wsplit: W = hi+lo
    "f32":    dict(dt=F32,  xsplit=False, wsplit=False),
    "bf16":   dict(dt=BF16, xsplit=False, wsplit=False),
    "fp16":   dict(dt=F16,  xsplit=False, wsplit=False),
    "fp16s2": dict(dt=F16,  xsplit=False, wsplit=True),
    "split3": dict(dt=BF16, xsplit=True,  wsplit=True),
    "fp16s3": dict(dt=F16,  xsplit=True,  wsplit=True),
}


def _is_t2(mode):
    """T2 attention layout: only for 1-pass 16-bit modes."""
    return mode in ("fp16", "bf16")


# ---------------------------------------------------------------------------
# kernel builder
# ---------------------------------------------------------------------------

def _build(mode, use_mask, use_qbias, use_kbias, use_vbias, use_ln_affine, reps=1):
    """Build + compile the SPMD single-core program. Returns (nc, input_names).

    reps>1 repeats the whole computation (idempotent output writes) so HW
    kernel time can be measured as the slope over reps."""
    nc = bacc.Bacc(
        "TRN2", target_bir_lowering=False, debug=False,
        enable_asserts=False, num_devices=NCORES,
    )

    xsplit, wsplit = MODES[mode]["xsplit"], MODES[mode]["wsplit"]
    wdt = MODES[mode]["dt"]
    xdt = wdt
    t2 = _is_t2(mode)
    # attention matmul operand dtype
    adt = wdt if t2 else (F16 if mode == "fp16" else F32)

    # ---------------- DRAM I/O ----------------
    names = []

    def dram_in(name, shape, dt):
        names.append(name)
        return nc.dram_tensor(name, shape, dt, kind="ExternalInput").ap()

    xt_hi = dram_in("xt_hi", [H, TPC], xdt)            # x^T (hi part if split)
    xt_lo = dram_in("xt_lo", [H, TPC], xdt) if xsplit else None
    xres = dram_in("xres", [TPC, H], F32)              # x + bo (residual input)
    w_dram = {"wq": dram_in("wq_hi", [H, H], wdt),
              "wk": dram_in("wk_hi", [H, H], wdt),
              "wv": dram_in("wv_hi", [H, H], wdt),
              "wo": dram_in("wo_hi", [H, H], wdt)}
    if wsplit:
        for wn in ("wq", "wk", "wv", "wo"):
            w_dram[wn + "_lo"] = dram_in(wn + "_lo", [H, H], wdt)
    bq = dram_in("bq", [128, NG], F32) if use_qbias else None   # (bq/8) chunked
    bk = dram_in("bk", [128, NG], F32) if use_kbias else None
    bvb = dram_in("bvb", [128, H], F32) if use_vbias else None  # bv broadcast
    gmb = dram_in("gmb", [128, H], F32) if use_ln_affine else None
    btb = dram_in("btb", [128, H], F32) if use_ln_affine else None
    mbias = dram_in("mbias", [CPC, W, W], F32) if use_mask else None
    out = nc.dram_tensor("out", [TPC, H], F32, kind="ExternalOutput").ap()

    # matmul pass list: (x-half, weight-key)
    if xsplit and wsplit:
        passes = {wn: [("hi", wn), ("hi", wn + "_lo"), ("lo", wn)]
                  for wn in ("wq", "wk", "wv", "wo")}
    elif wsplit:
        passes = {wn: [("hi", wn), ("hi", wn + "_lo")]
                  for wn in ("wq", "wk", "wv", "wo")}
    else:
        passes = {wn: [("hi", wn)] for wn in ("wq", "wk", "wv", "wo")}

    with tile.TileContext(nc) as tc, ExitStack() as ctx:
        const = ctx.enter_context(tc.tile_pool(name="const", bufs=1))
        xp = ctx.enter_context(tc.tile_pool(name="xp", bufs=2))
        qkv = ctx.enter_context(tc.tile_pool(name="qkv", bufs=1))
        attn = ctx.enter_context(tc.tile_pool(name="attn", bufs=3))
        ctxnp = ctx.enter_context(tc.tile_pool(name="ctxnp", bufs=2))
        ctxp = ctx.enter_context(tc.tile_pool(name="ctxp", bufs=1))
        outp = ctx.enter_context(tc.tile_pool(name="outp", bufs=2))
        # single PSUM pool: every tile = one bank, one shared tag, 8-deep
        # rotation => concurrent quadrant matmuls always hit distinct banks
        pp = ctx.enter_context(tc.tile_pool(name="pp", bufs=8, space="PSUM"))

        def ptile(shape, dt, name=None):
            return pp.tile(shape, dt, tag="ps", name=name)

        # ---------------- constants ----------------
        ident = const.tile([128, 128], adt, tag="ident")
        make_identity(nc, ident)

        w_sb = {}   # key -> list of NG chunk tiles [128, H]
        for wn, ap in w_dram.items():
            w_sb[wn] = []
            for g in range(NG):
                t = const.tile([128, H], wdt, tag=f"{wn}{g}")
                nc.sync.dma_start(t[:], ap[g * 128:(g + 1) * 128, :])
                w_sb[wn].append(t)

        bq_sb = bk_sb = bvb_sb = gmb_sb = btb_sb = None
        if use_qbias:
            bq_sb = const.tile([128, NG], F32, tag="bq")
            nc.sync.dma_start(bq_sb[:], bq)
        if use_kbias:
            bk_sb = const.tile([128, NG], F32, tag="bk")
            nc.sync.dma_start(bk_sb[:], bk)
        if use_vbias:
            bvb_sb = const.tile([128, H], F32, tag="bvb")
            nc.sync.dma_start(bvb_sb[:], bvb)
        if use_ln_affine:
            gmb_sb = const.tile([128, H], F32, tag="gmb")
            nc.sync.dma_start(gmb_sb[:], gmb)
            btb_sb = const.tile([128, H], F32, tag="btb")
            nc.sync.dma_start(btb_sb[:], btb)

        # ---------------- per token-block ----------------
        # reps>1: device-side hardware loop repeating the whole computation
        # (for slope-based HW timing); body is identical each iteration.
        import contextlib
        rep_cm = tc.For_i(0, reps, 1) if reps > 1 else contextlib.nullcontext()
        with rep_cm:
          for blk in range(NBLK):
            t0 = blk * BLK

            # -- load x^T block --
            xth = [xp.tile([128, BLK], xdt, tag=f"xth{g}", name=f"xth{g}") for g in range(NG)]
            for g in range(NG):
                nc.sync.dma_start(xth[g][:], xt_hi[g * 128:(g + 1) * 128, t0:t0 + BLK])
            if xsplit:
                xtl = [xp.tile([128, BLK], xdt, tag=f"xtl{g}", name=f"xtl{g}") for g in range(NG)]
                for g in range(NG):
                    nc.sync.dma_start(xtl[g][:], xt_lo[g * 128:(g + 1) * 128, t0:t0 + BLK])

            def xop(sel, g):
                return xth[g] if sel == "hi" else xtl[g]

            # -- Q/K projections (transposed layout [h_out, t]) --
            qT, kT = [], []
            for which, wn, dst in (("q", "wq", qT), ("k", "wk", kT)):
                for go in range(NG):
                    ps = ptile([128, BLK], F32, name="proj")
                    mms = [(wkey, gi, xsel) for xsel, wkey in passes[wn] for gi in range(NG)]
                    for i, (wkey, gi, xsel) in enumerate(mms):
                        nc.tensor.matmul(
                            ps[:],
                            w_sb[wkey][gi][:, go * 128:(go + 1) * 128],
                            xop(xsel, gi)[:],
                            start=(i == 0), stop=(i == len(mms) - 1),
                        )
                    sb = qkv.tile([128, BLK], adt, tag=f"{which}T{go}")
                    scl = 0.125 if which == "q" else 1.0
                    has_b = use_qbias if which == "q" else use_kbias
                    if has_b:
                        bias = (bq_sb if which == "q" else bk_sb)[:, go:go + 1]
                        nc.scalar.activation(sb[:], ps[:], FP.Identity, bias=bias, scale=scl)
                    else:
                        nc.scalar.activation(sb[:], ps[:], FP.Copy, scale=scl)
                    dst.append(sb)

            # -- V projection --
            vN = []
            if t2:
                # natural layout in 12 groups of 65 cols (64 dims + ones col)
                for tt in range(CPB):
                    vt = qkv.tile([128, NH * 65], adt, tag=f"v{tt}")
                    v3 = vt[:].rearrange("p (g c) -> p g c", c=65)
                    nc.vector.memset(v3[:, :, 64:65], 1.0)
                    # nhalf inner so consecutive matmuls share the stationary
                    # operand (the x tile) -> redundant LDWEIGHTS can elide
                    psv = [ptile([128, 384], F32, name="proj") for _ in range(2)]
                    mms = [(wkey, gi, xsel) for xsel, wkey in passes["wv"] for gi in range(NG)]
                    for i, (wkey, gi, xsel) in enumerate(mms):
                        for nhalf in range(2):
                            nc.tensor.matmul(
                                psv[nhalf][:],
                                xop(xsel, gi)[:, tt * 128:(tt + 1) * 128],
                                w_sb[wkey][gi][:, nhalf * 384:(nhalf + 1) * 384],
                                start=(i == 0), stop=(i == len(mms) - 1),
                            )
                    for nhalf in range(2):
                        dstv = v3[:, 6 * nhalf:6 * nhalf + 6, 0:64]
                        srcv = psv[nhalf][:].rearrange("p (g c) -> p g c", c=64)
                        if use_vbias:
                            bsrc = bvb_sb[:, nhalf * 384:(nhalf + 1) * 384].rearrange("p (g c) -> p g c", c=64)
                            nc.vector.tensor_add(dstv, srcv, bsrc)
                        else:
                            nc.vector.tensor_copy(dstv, srcv)
                    vN.append(vt)
            else:
                for tt in range(CPB):
                    vt = qkv.tile([128, H], adt, tag=f"v{tt}")
                    for nhalf in range(2):
                        n0 = nhalf * 384
                        ps = ptile([128, 384], F32, name="proj")
                        mms = [(wkey, gi, xsel) for xsel, wkey in passes["wv"] for gi in range(NG)]
                        for i, (wkey, gi, xsel) in enumerate(mms):
                            nc.tensor.matmul(
                                ps[:],
                                xop(xsel, gi)[:, tt * 128:(tt + 1) * 128],
                                w_sb[wkey][gi][:, n0:n0 + 384],
                                start=(i == 0), stop=(i == len(mms) - 1),
                            )
                        if use_vbias:
                            nc.vector.tensor_add(vt[:, n0:n0 + 384], ps[:], bvb_sb[:, n0:n0 + 384])
                        else:
                            nc.vector.tensor_copy(vt[:, n0:n0 + 384], ps[:])
                    vN.append(vt)

            # -- block-diagonal attention --
            cxdt = F32 if mode == "f32" else wdt
            if t2:
                # cxall [128, NG*BLK]: head-pair g occupies cols g*BLK..(g+1)*BLK
                cxall = ctxp.tile([128, NG * BLK], cxdt, tag="cxall")
                cxh = None
            else:
                cxh = [ctxp.tile([128, BLK], cxdt, tag=f"cxh{g}", name=f"cxh{g}") for g in range(NG)]
            cxl = [ctxp.tile([128, BLK], wdt, tag=f"cxl{g}", name=f"cxl{g}") for g in range(NG)] if wsplit else None

            if t2:
                # ---- T2 path: transposed scores, natural-ctx PV, software-
                # pipelined two chunks deep so PE never waits on the in-order
                # ACT/DVE queues:
                #   iter cc: [pass2(cc-1), pass1(cc), transpose(cc-1),
                #             out-proj+LN(cc-2)]
                def pass1(cc):
                    ts = slice(cc * 128, (cc + 1) * 128)
                    if use_mask:
                        mb = attn.tile([128, W], F32, tag="mb")
                        nc.sync.dma_start(mb[:], mbias[blk * CPB + cc])
                    exs = []
                    for g in range(NPAIR):
                        ps_s = [ptile([128, W], F32, name=f"scps{h}")
                                for h in range(2)]
                        for h in range(2):
                            p0 = h * 64
                            # scoresT[k, q] = kT_h^T @ qT_h
                            nc.tensor.matmul(
                                ps_s[h][:],
                                kT[g][p0:p0 + 64, ts],
                                qT[g][p0:p0 + 64, ts],
                                start=True, stop=True,
                                tile_position=(p0, 0),
                            )
                        ex = attn.tile([128, 2 * W], adt, tag="ex", bufs=12)
                        for h in range(2):
                            src = ps_s[h][:]
                            if use_mask:
                                sm = attn.tile([128, W], F32, tag="sm")
                                nc.vector.tensor_add(sm[:], src, mb[:])
                                src = sm[:]
                            nc.scalar.activation(ex[:, h * W:(h + 1) * W], src, FP.Exp)
                        exs.append(ex)
                    return exs

                def pass2(cc, exs):
                    ctxn = ctxnp.tile([128, H], adt, tag="ctxn")
                    for g in range(NPAIR):
                        # PV natural: pc[q, 65] per head; col 64 = denominator
                        pc = ptile([128, 130], F32, name="pc")
                        for h in range(2):
                            hh = 2 * g + h
                            nc.tensor.matmul(
                                pc[:, h * 65:(h + 1) * 65],
                                exs[g][:, h * W:(h + 1) * W],
                                vN[cc][:, hh * 65:(hh + 1) * 65],
                                start=True, stop=True,
                                skip_group_check=(h == 1),
                            )
                        rc = attn.tile([128, 2], F32, tag="rc")
                        den3 = pc[:].rearrange("p (h c) -> p h c", c=65)[:, :, 64:65]
                        rc3 = rc[:].rearrange("p (h o) -> p h o", o=1)
                        nc.vector.reciprocal(rc3, den3)
                        # normalize: head 0 on DVE, head 1 on ACT (balance)
                        nc.vector.tensor_scalar_mul(
                            ctxn[:, (2 * g) * 64:(2 * g + 1) * 64],
                            pc[:, 0:64], rc[:, 0:1])
                        nc.scalar.activation(
                            ctxn[:, (2 * g + 1) * 64:(2 * g + 2) * 64],
                            pc[:, 65:129], FP.Identity, scale=rc[:, 1:2])
                    return ctxn

                def do_tp(cc, ctxn):
                    ts = slice(cc * 128, (cc + 1) * 128)
                    # transpose natural ctx back to [hd, t]: 6 transposes into
                    # one fp16 PSUM bank, then one strided DVE copy
                    tp = ptile([128, H], adt, name="tp")
                    for g in range(NG):
                        nc.tensor.matmul(
                            tp[:, g * 128:(g + 1) * 128],
                            ctxn[:, g * 128:(g + 1) * 128],
                            ident[:], is_transpose=True,
                            skip_group_check=(g > 0),
                        )
                    dstc = cxall[:].rearrange("p (g t) -> p g t", g=NG)[:, :, ts]
                    srcc = tp[:].rearrange("p (g c) -> p g c", c=128)
                    nc.vector.tensor_copy(dstc, srcc)
            else:
              for cc in range(CPB):
                ts = slice(cc * 128, (cc + 1) * 128)
                chunk_idx = blk * CPB + cc
                if use_mask:
                    mb = attn.tile([128, W], F32, tag="mb")
                    nc.sync.dma_start(mb[:], mbias[chunk_idx])
                if True:
                    # ---- legacy path: scores [q, k], PE prob transpose ----
                    for g in range(NPAIR):
                        ps_s = [ptile([128, W], F32, name=f"scps{h}")
                                for h in range(2)]
                        for h in range(2):
                            p0 = h * 64
                            nc.tensor.matmul(
                                ps_s[h][:],
                                qT[g][p0:p0 + 64, ts],
                                kT[g][p0:p0 + 64, ts],
                                start=True, stop=True,
                                tile_position=(p0, 0),
                            )
                        den = attn.tile([128, 2], F32, tag="den")
                        ex = attn.tile([128, 2 * W], F32, tag="ex")
                        for h in range(2):
                            src = ps_s[h][:]
                            if use_mask:
                                sm = attn.tile([128, W], F32, tag="sm")
                                nc.vector.tensor_add(sm[:], src, mb[:])
                                src = sm[:]
                            nc.scalar.activation(
                                ex[:, h * W:(h + 1) * W], src, FP.Exp,
                                accum_out=den[:, h:h + 1],
                            )
                        rec = attn.tile([128, 2], F32, tag="rec")
                        nc.vector.reciprocal(rec[:], den[:])
                        pr = attn.tile([128, 2 * W], adt, tag="pr")
                        for h in range(2):
                            nc.vector.tensor_scalar_mul(
                                pr[:, h * W:(h + 1) * W], ex[:, h * W:(h + 1) * W],
                                rec[:, h:h + 1],
                            )
                        ps_t = ptile([128, 2 * W], adt, name="pt")
                        for h in range(2):
                            nc.tensor.matmul(
                                ps_t[:, h * W:(h + 1) * W], pr[:, h * W:(h + 1) * W],
                                ident[:], is_transpose=True,
                                skip_group_check=(h == 1),
                            )
                        pts = attn.tile([128, 2 * W], adt, tag="pts")
                        nc.vector.tensor_copy(pts[:], ps_t[:])
                        ps_c = ptile([128, W], F32, name="cx")
                        for h in range(2):
                            hd0 = (2 * g + h) * HD
                            nc.tensor.matmul(
                                ps_c[h * 64:(h + 1) * 64, :],
                                vN[cc][:, hd0:hd0 + HD],
                                pts[:, h * W:(h + 1) * W],
                                start=True, stop=True,
                                tile_position=(0, h * 64),
                                skip_group_check=(h == 1),
                            )
                        if wsplit:
                            nc.scalar.activation(cxh[g][:, ts], ps_c[:], FP.Copy)
                            nc.vector.tensor_sub(cxl[g][:, ts], ps_c[:], cxh[g][:, ts])
                        else:
                            nc.vector.tensor_copy(cxh[g][:, ts], ps_c[:])

            # -- output projection + residual + LayerNorm --
            if t2:
                def olhs(pi, gi, tt):
                    return cxall[:, gi * BLK + tt * 128:gi * BLK + (tt + 1) * 128]
                opasses = [0]
            else:
                if wsplit:
                    ocx = [(cxh, "wo"), (cxh, "wo_lo"), (cxl, "wo")]
                else:
                    ocx = [(cxh, "wo")]

                def olhs(pi, gi, tt):
                    return ocx[pi][0][gi][:, tt * 128:(tt + 1) * 128]
                opasses = list(range(len(ocx)))

            def owkey(pi):
                if t2:
                    return "wo"
                return ocx[pi][1]

            def oproj_ln(tt):
                r0 = t0 + tt * 128
                xr = outp.tile([128, H], F32, tag="xr")
                nc.sync.dma_start(xr[:], xres[r0:r0 + 128, :])
                hsb = outp.tile([128, H], F32, tag="hsb")
                # nhalf inner so consecutive matmuls share the stationary
                # operand (the ctx slice) -> redundant LDWEIGHTS can elide
                pso = [ptile([128, 384], F32, name="ops") for _ in range(2)]
                mms = [(pi, gi) for pi in opasses for gi in range(NG)]
                for i, (pi, gi) in enumerate(mms):
                    for nhalf in range(2):
                        nc.tensor.matmul(
                            pso[nhalf][:],
                            olhs(pi, gi, tt),
                            w_sb[owkey(pi)][gi][:, nhalf * 384:(nhalf + 1) * 384],
                            start=(i == 0), stop=(i == len(mms) - 1),
                        )
                for nhalf in range(2):
                    n0 = nhalf * 384
                    nc.vector.tensor_add(hsb[:, n0:n0 + 384], pso[nhalf][:], xr[:, n0:n0 + 384])

                # LayerNorm stats via bn_stats/bn_aggr
                st = outp.tile([128, 12], F32, tag="st")
                for nhalf in range(2):
                    nc.vector.bn_stats(st[:, nhalf * 6:(nhalf + 1) * 6],
                                       hsb[:, nhalf * 384:(nhalf + 1) * 384])
                mv = outp.tile([128, 2], F32, tag="mv")
                nc.vector.bn_aggr(mv[:], st[:])
                var1 = outp.tile([128, 1], F32, tag="var1")
                nc.vector.tensor_scalar_add(var1[:], mv[:, 1:2], EPS)
                # rstd = 1/sqrt(var): bit-trick seed + 2 Newton steps (on DVE,
                # avoiding the ACT sqrt table-set switch and its poor ULP)
                rstd = outp.tile([128, 1], F32, tag="rstd")
                t1 = outp.tile([128, 1], F32, tag="t1n")
                ri = rstd[:].bitcast(mybir.dt.int32)
                nc.vector.tensor_scalar(
                    ri, var1[:].bitcast(mybir.dt.int32), 1, None,
                    op0=OP.logical_shift_right,
                )
                nc.vector.tensor_scalar(ri, ri, -1, 0x5F3759DF, op0=OP.mult, op1=OP.add)
                for _ in range(2):
                    nc.vector.tensor_mul(t1[:], rstd[:], rstd[:])
                    nc.vector.tensor_mul(t1[:], t1[:], var1[:])
                    nc.vector.tensor_scalar(t1[:], t1[:], -0.5, 1.5, op0=OP.mult, op1=OP.add)
                    nc.vector.tensor_mul(rstd[:], rstd[:], t1[:])
                # apply on ACT: out = (h - mu) * rstd = h*rstd + (-mu*rstd)
                nmr = outp.tile([128, 1], F32, tag="nmr")
                nc.vector.tensor_scalar(nmr[:], mv[:, 0:1], rstd[:], -1.0,
                                        op0=OP.mult, op1=OP.mult)
                ot = outp.tile([128, H], F32, tag="ot")
                for nhalf in range(2):
                    n0 = nhalf * 384
                    nc.scalar.activation(ot[:, n0:n0 + 384], hsb[:, n0:n0 + 384],
                                         FP.Identity, bias=nmr[:], scale=rstd[:])
                if use_ln_affine:
                    nc.vector.tensor_mul(ot[:], ot[:], gmb_sb[:])
                    nc.vector.tensor_add(ot[:], ot[:], btb_sb[:])
                nc.sync.dma_start(out[r0:r0 + 128, :], ot[:])

            if t2:
                exs_prev = None
                ctxn_prev = None
                for cc in range(CPB):
                    if exs_prev is not None:
                        ctxn_prev = pass2(cc - 1, exs_prev)
                    exs_new = pass1(cc)
                    if exs_prev is not None:
                        do_tp(cc - 1, ctxn_prev)
                    if cc >= 2:
                        oproj_ln(cc - 2)
                    exs_prev = exs_new
                ctxn_prev = pass2(CPB - 1, exs_prev)
                do_tp(CPB - 1, ctxn_prev)
                oproj_ln(CPB - 2)
                oproj_ln(CPB - 1)
            else:
                for tt in range(CPB):
                    oproj_ln(tt)

    nc.compile()
    return nc, names


# ---------------------------------------------------------------------------
# host-side wrapper
# ---------------------------------------------------------------------------

_CACHE = {}


def _get_program(mode, use_mask, use_qbias, use_kbias, use_vbias, use_ln_affine, reps=1):
    key = (mode, use_mask, use_qbias, use_kbias, use_vbias, use_ln_affine, reps)
    if key not in _CACHE:
        _CACHE[key] = _build(*key[:-1], reps=reps)
    return _CACHE[key]


def _prep_inputs(inputs, mode):
    """Host preprocessing -> per-core in_maps + program flags."""
    hs = np.ascontiguousarray(np.asarray(inputs["hidden_states"], dtype=np.float32))
    mask = np.asarray(inputs["attention_mask"], dtype=np.float32)
    Wq = np.asarray(inputs["Wq"], np.float32); bq = np.asarray(inputs["bq"], np.float32)
    Wk = np.asarray(inputs["Wk"], np.float32); bk = np.asarray(inputs["bk"], np.float32)
    Wv = np.asarray(inputs["Wv"], np.float32); bv = np.asarray(inputs["bv"], np.float32)
    Wo = np.asarray(inputs["Wo"], np.float32); bo = np.asarray(inputs["bo"], np.float32)
    gm = np.asarray(inputs["ln_gamma"], np.float32)
    bt = np.asarray(inputs["ln_beta"], np.float32)

    cfg = MODES[mode]
    xsplit, wsplit = cfg["xsplit"], cfg["wsplit"]
    npdt = {F32: np.float32, BF16: ml_dtypes.bfloat16, F16: np.float16}[cfg["dt"]]
    use_mask = not np.all(mask == 1.0)
    use_qbias = bool(np.any(bq)); use_kbias = bool(np.any(bk))
    use_vbias = bool(np.any(bv))
    use_ln_affine = bool(np.any(gm != 1.0) or np.any(bt))

    x = hs.reshape(B * S, H)
    xres_full = x + bo[None, :] if np.any(bo) else x

    def wpack(w):
        wh = w.astype(npdt)
        d = {"hi": np.ascontiguousarray(wh)}
        if wsplit:
            d["lo"] = np.ascontiguousarray((w - wh.astype(np.float32)).astype(npdt))
        return d

    wq, wk, wv, wo = wpack(Wq), wpack(Wk), wpack(Wv), wpack(Wo)

    if use_mask:
        # per-core diagonal [W,W] blocks of the mask -> additive bias
        m4 = mask.reshape(B, C, W, C, W)
        idx = np.arange(C)
        mblk = m4[:, idx, :, idx, :]                 # [C,B,W,W]
        mblk = np.transpose(mblk, (1, 0, 2, 3))      # [B,C,W,W]
        bias_blocks = ((1.0 - mblk) * NEG).astype(np.float32).reshape(B * C, W, W)
        if _is_t2(mode):
            # T2 computes scores transposed ([k, q]) -> transpose the bias
            bias_blocks = np.ascontiguousarray(np.transpose(bias_blocks, (0, 2, 1)))

    in_maps = []
    for c in range(NCORES):
        sl = x[c * TPC:(c + 1) * TPC]                # [TPC, H]
        m = {}
        xh = sl.astype(npdt)
        m["xt_hi"] = np.ascontiguousarray(xh.T)
        if xsplit:
            m["xt_lo"] = np.ascontiguousarray((sl - xh.astype(np.float32)).astype(npdt).T)
        m["xres"] = np.ascontiguousarray(xres_full[c * TPC:(c + 1) * TPC])
        for wn, d in (("wq", wq), ("wk", wk), ("wv", wv), ("wo", wo)):
            m[wn + "_hi"] = d["hi"]
            if wsplit:
                m[wn + "_lo"] = d["lo"]
        if use_qbias:
            m["bq"] = np.ascontiguousarray((bq / 8.0).reshape(NG, 128).T)
        if use_kbias:
            m["bk"] = np.ascontiguousarray(bk.reshape(NG, 128).T)
        if use_vbias:
            m["bvb"] = np.ascontiguousarray(np.broadcast_to(bv, (128, H)))
        if use_ln_affine:
            m["gmb"] = np.ascontiguousarray(np.broadcast_to(gm, (128, H)))
            m["btb"] = np.ascontiguousarray(np.broadcast_to(bt, (128, H)))
        if use_mask:
            m["mbias"] = np.ascontiguousarray(bias_blocks[c * CPC:(c + 1) * CPC])
        in_maps.append(m)

    flags = (use_mask, use_qbias, use_kbias, use_vbias, use_ln_affine)
    return in_maps, flags


def run(inputs, mode=None, trace=False, reps=1):
    """Run the kernel; returns (output [B,S,H] f32, BassKernelResults)."""
    mode = mode or MODE
    in_maps, flags = _prep_inputs(inputs, mode)
    nc, names = _get_program(mode, *flags, reps=reps)
    in_maps = [{k: v for k, v in m.items() if k in names} for m in in_maps]
    res = run_bass_kernel_spmd(nc, in_maps, list(range(NCORES)), trace=trace)
    outs = [res.results[c]["out"] for c in range(NCORES)]
    full = np.concatenate(outs, axis=0).reshape(B, S, H).astype(np.float32)
    return full, res


def kernel(**inputs):
    out, _ = run(inputs)
    return out
